# revision 42
# baseline (speedup 1.0000x reference)
"""Trainium2 Bass kernel for nn_End2EndRVFixedOutput (nms_detection).

Reference semantics: out[100,7] starts at zeros; for n = 0..7 in order,
with off_n = (0 if n==0 else num_dets[n-1]) and k_n = num_dets[n],
rows [off_n, off_n+k_n) are overwritten with
[n, boxes[n,j,0:4], classes[n,j], scores[n,j]] for j = row-off_n.

num_dets < 12, so only the [:, :12] input slices matter and only out rows
0..21 can ever be written.  Device algorithm (per core, inputs replicated):

  1. x7[96,7] = [vd | boxes | classes | scores] for rows p = 12n+j is
     assembled by direct column DMAs straight from the full DRAM tensors;
     a plain DMA zero-fills the output (ordered before the scatter by an
     explicit dependency).
  2. num_dets is cast and partition-shifted (stream_shuffle) to give per-
     batch k and off; tiny bf16 matmuls against selection constants
     broadcast them to the 96 (n,j) rows and compute, per output row r,
     batch coverage rm8[n,r] = (off_n <= r < off_n+k_n) and its suffix
     count stn[n,r] = sum_{m>n} rm8[m,r] (packed as one PSUM tile
     [stn | 4096*rm]).  Scatter targets and the last-writer gate:
        rpv[p]   = off_n + j + 1e6 + 1e6*(j >= k_n)
        a96c[p]  = stn96[p,r_p] + 4096*rm96[p,r_p]   # one-hot + accum_out
        w96[p]   = (a96c[p] == 4096)                 # covered, no later writer
        ridx[p]  = rpv[p] - 1e6*w96[p]
  3. One indirect DMA scatters x7 rows to out[ridx].  Gating makes the
     destinations UNIQUE (exactly the winning writer per row), so nothing
     relies on DMA descriptor ordering; indices >= 1e6 are skipped via
     bounds_check, leaving those rows at the zero-fill value.

All arithmetic is exact (masks are 0/1, indices are small ints), so the
output matches the reference bit-for-bit.  Every core runs the full
(tiny) computation; core 0's output is returned.  Measured on trn2:
~17.5 us HW exec per core (vs ~13.5 us for an empty DMA-through kernel
on this stack), relative error 0.0.
"""

import sys

import numpy as np

_TRN_REPO = "/opt/trn_rl_repo"
if _TRN_REPO not in sys.path:
    sys.path.insert(0, _TRN_REPO)

import ml_dtypes

import concourse.bacc as bacc
import concourse.bass as bass
import concourse.mybir as mybir
import concourse.tile as tile
from concourse.bass_utils import run_bass_kernel_spmd

B = 8          # batches
N_FULL = 8192  # detections per batch in the full input
J = 12         # num_dets < 12, so only rows [:12] of each batch matter
R = 100        # fixed output rows
P96 = B * J    # 96 stacked (batch, j) rows
OOB = 1.0e6    # pushed past bounds_check so the scatter skips the row

F32 = mybir.dt.float32
BF16 = mybir.dt.bfloat16
I32 = mybir.dt.int32

# f32 constant blob CB96 [96,3] = j96 | j96+OOB | vd96
CONST_LEN = P96 * 3
# bf16 constant blob: U96 | SEL96 | 4096*SEL96, packed per-row as [8,288]
GW = 4096.0  # weight separating the rm-half from the stn-half in the accum
CONSTBF_LEN = 8 * (3 * P96)


def _make_consts():
    p = np.arange(P96)
    m = np.arange(B)
    j96 = (p % J).astype(np.float32)[:, None]                            # [96,1]
    vd96 = (p // J).astype(np.float32)[:, None]                          # [96,1]
    blob = (
        np.concatenate([j96, j96 + OOB, vd96], axis=1).ravel().astype(np.float32)
    )
    assert blob.shape == (CONST_LEN,)
    u96 = (m[:, None] > p[None, :] // J).astype(np.float32)              # [8,96]
    sel96 = (m[:, None] == p[None, :] // J).astype(np.float32)           # [8,96]
    blobbf = (
        np.concatenate([u96, sel96, GW * sel96], axis=1)
        .ravel()
        .astype(ml_dtypes.bfloat16)
    )
    assert blobbf.shape == (CONSTBF_LEN,)
    return np.ascontiguousarray(blob), np.ascontiguousarray(blobbf)


def _build_nc() -> bass.Bass:
    nc = bacc.Bacc(None, target_bir_lowering=False)
    nd_d = nc.dram_tensor("num_dets", [B], I32, kind="ExternalInput")
    boxes_d = nc.dram_tensor("boxes", [B, N_FULL, 4], F32, kind="ExternalInput")
    scores_d = nc.dram_tensor("scores", [B, N_FULL], F32, kind="ExternalInput")
    classes_d = nc.dram_tensor("classes", [B, N_FULL], F32, kind="ExternalInput")
    const_d = nc.dram_tensor("consts", [CONST_LEN], F32, kind="ExternalInput")
    constbf_d = nc.dram_tensor("constsbf", [CONSTBF_LEN], BF16, kind="ExternalInput")
    out_d = nc.dram_tensor("out", [R, 7], F32, kind="ExternalOutput")

    with tile.TileContext(nc) as tc:
        with (
            tc.tile_pool(name="sb", bufs=1) as sb,
            tc.tile_pool(name="ps", bufs=1, space=bass.MemorySpace.PSUM) as ps,
        ):
            ndi = sb.tile([B, 1], I32)
            cb96 = sb.tile([P96, 3], F32)
            r8i = sb.tile([B, R], I32)
            r2i = sb.tile([P96, 2 * R], I32)
            usel = sb.tile([B, 3 * P96], BF16)
            x7 = sb.tile([P96, 7], F32)
            z7 = sb.tile([R, 7], F32)

            k32 = sb.tile([32, 1], F32)
            off32 = sb.tile([32, 1], F32)
            k8bf = sb.tile([B, 1], BF16)
            off8bf = sb.tile([B, 1], BF16)
            s8f = sb.tile([B, 1], F32)
            u8c = sb.tile([B, R], F32)
            rm8 = sb.tile([B, R], BF16)
            b2 = sb.tile([P96, 1], F32)
            rpv = sb.tile([P96, 1], F32)
            scr200 = sb.tile([P96, 2 * R], F32)
            a96c = sb.tile([P96, 1], F32)
            w96 = sb.tile([P96, 1], F32)
            ridx = sb.tile([P96, 1], I32)

            comb = ps.tile([P96, 2 * R], F32)
            k96p = ps.tile([P96, 1], F32)
            off96p = ps.tile([P96, 1], F32)

            U96 = usel[:, 0:P96]
            SEL96 = usel[:, P96 : 2 * P96]
            SEL96W = usel[:, 2 * P96 : 3 * P96]
            J96 = cb96[:, 0:1]
            JO96 = cb96[:, 1:2]
            VD96 = cb96[:, 2:3]

            nc.gpsimd.memset(k32[:], 0.0)
            nc.gpsimd.memset(z7[:], 0.0)
            # on-device iotas replace the big row-index constants:
            # r8i[n,r] = r; r2i[p,:] = [r+OOB | r+OOB] (both accum halves)
            nc.gpsimd.iota(r8i[:], pattern=[[1, R]], base=0, channel_multiplier=0)
            nc.gpsimd.iota(
                r2i[:], pattern=[[0, 2], [1, R]], base=int(OOB), channel_multiplier=0
            )

            # loads on the two HWDGE queues only (gpsimd is kept free for the
            # indirect scatter); critical ones first.  The zero-fill of the
            # output is a plain DMA; an explicit dep orders it before the
            # scatter.
            nc.sync.dma_start(out=ndi[:], in_=nd_d[:].rearrange("(p f) -> p f", f=1))
            zfill = nc.gpsimd.dma_start(out=out_d[:], in_=z7[:])
            nc.sync.dma_start(out=x7[:, 1:5], in_=boxes_d[:, 0:J, :])
            nc.sync.dma_start(out=x7[:, 5:6], in_=classes_d[:, 0:J])
            nc.scalar.dma_start(
                out=cb96[:], in_=const_d[:].rearrange("(p f) -> p f", p=P96)
            )
            nc.scalar.dma_start(out=usel[:], in_=constbf_d[:].rearrange(
                "(p f) -> p f", p=B
            ))
            nc.gpsimd.dma_start(out=x7[:, 6:7], in_=scores_d[:, 0:J])

            alu = mybir.AluOpType
            vec = nc.vector

            # k32[0:8] = float(num_dets); off32[n] = k32[n-1] via partition shift
            vec.tensor_copy(k32[0:B, :], ndi[:])
            vec.tensor_copy(k8bf[:], k32[0:B, :])
            vec.stream_shuffle(off32[:], k32[:], mask=[31] + list(range(31)))
            vec.tensor_copy(off8bf[:], off32[0:B, :])
            # vd column of x7 (scalar engine: DVE is the busy one)
            nc.scalar.copy(x7[:, 0:1], VD96)
            # broadcast k and off to the 96 (n,j) rows via tiny matmuls
            nc.tensor.matmul(k96p[:], SEL96, k8bf[:], start=True, stop=True)
            nc.tensor.matmul(off96p[:], SEL96, off8bf[:], start=True, stop=True)
            # batch coverage masks on 8 partitions
            vec.tensor_tensor(s8f[:], k32[0:B, :], off32[0:B, :], alu.add)
            vec.tensor_scalar(u8c[:], r8i[:], off32[0:B, :], None, alu.is_ge)
            vec.scalar_tensor_tensor(
                rm8[:], r8i[:], s8f[:], u8c[:], alu.is_lt, alu.mult
            )
            # two parallel matmuls into one PSUM tile: cols 0:100 hold
            # stn96[p,r] = sum_{m>n} rm8[m,r], cols 100:200 hold GW*rm8[n,r]
            nc.tensor.matmul(comb[:, 0:R], U96, rm8[:], start=True, stop=True)
            nc.tensor.matmul(comb[:, R : 2 * R], SEL96W, rm8[:], start=True, stop=True)

            # per-(n,j) scatter targets (fills DVE gaps while PE runs);
            # rpv = off + j + OOB + OOB*(j >= k)
            vec.tensor_scalar(b2[:], k96p[:], J96, OOB, alu.is_le, alu.mult)
            rpv_op = vec.scalar_tensor_tensor(
                rpv[:], off96p[:], JO96, b2[:], alu.add, alu.add
            )

            # one-hot extraction of both halves at r+OOB = rpv[p]:
            # a96c[p] = stn96[p,r_p] + GW*rm96[p,r_p]; winner iff == GW
            vec.scalar_tensor_tensor(
                scr200[:], r2i[:], rpv[:], comb[:], alu.is_equal, alu.mult,
                accum_out=a96c[:],
            )
            vec.tensor_scalar(w96[:], a96c[:], GW, None, alu.is_equal)
            # ridx = rpv - OOB*w96: winners land on their row, rest stay OOB
            vec.scalar_tensor_tensor(
                ridx[:], w96[:], -OOB, rpv[:], alu.mult, alu.add
            )

            # wake gpsimd early (on rpv) so its sequencer is already parked
            # at the scatter's own wait when ridx posts
            wk = nc.gpsimd.engine_nop()
            bass._add_dep_helper(wk.ins, rpv_op.ins, sync=True, reason="early wake")

            # winner-only scatter: destinations are unique, no ordering needed
            scat = nc.gpsimd.indirect_dma_start(
                out=out_d[:],
                out_offset=bass.IndirectOffsetOnAxis(ap=ridx[:], axis=0),
                in_=x7[:],
                in_offset=None,
                bounds_check=R - 1,
                oob_is_err=False,
            )
            # the zero-fill must fully land before the data scatter
            bass._add_dep_helper(
                scat.ins, zfill.ins, sync=True, reason="zero-fill before scatter"
            )

    nc.finalize()
    return nc


_CACHE: dict = {}


def _get_built():
    if "nc" not in _CACHE:
        _CACHE["nc"] = _build_nc()
        _CACHE["consts"] = _make_consts()
    return _CACHE["nc"], _CACHE["consts"]


def run(inputs: dict, trace: bool = False, **spmd_kwargs):
    """Run on all 8 cores with replicated inputs; returns (out, BassKernelResults)."""
    nc, (consts, constsbf) = _get_built()
    in_map = {
        "num_dets": np.ascontiguousarray(inputs["num_dets"], dtype=np.int32),
        "boxes": np.ascontiguousarray(inputs["boxes"], dtype=np.float32),
        "scores": np.ascontiguousarray(inputs["scores"], dtype=np.float32),
        "classes": np.ascontiguousarray(inputs["classes"], dtype=np.float32),
        "consts": consts,
        "constsbf": constsbf,
    }
    res = run_bass_kernel_spmd(
        nc,
        [dict(in_map) for _ in range(8)],
        core_ids=list(range(8)),
        trace=trace,
        **spmd_kwargs,
    )
    return res.results[0]["out"], res


def kernel(num_dets, boxes, scores, classes):
    out, _ = run(
        {"num_dets": num_dets, "boxes": boxes, "scores": scores, "classes": classes}
    )
    return out


# revision 44
# speedup vs baseline: 1.0196x; 1.0196x over previous
"""Trainium2 Bass kernel for nn_End2EndRVFixedOutput (nms_detection).

Reference semantics: out[100,7] starts at zeros; for n = 0..7 in order,
with off_n = (0 if n==0 else num_dets[n-1]) and k_n = num_dets[n],
rows [off_n, off_n+k_n) are overwritten with
[n, boxes[n,j,0:4], classes[n,j], scores[n,j]] for j = row-off_n.

num_dets < 12, so only the [:, :12] input slices matter and only out rows
0..21 can ever be written.  Device algorithm (per core, inputs replicated):

  1. x7[96,7] = [vd | boxes | classes | scores] for rows p = 12n+j is
     assembled by direct column DMAs straight from the full DRAM tensors;
     a plain DMA zero-fills the output (ordered before the scatter by an
     explicit dependency).
  2. num_dets is cast and partition-shifted (stream_shuffle) to give per-
     batch k and off; tiny bf16 matmuls against selection constants
     broadcast them to the 96 (n,j) rows and compute, per output row r,
     batch coverage rm8[n,r] = (off_n <= r < off_n+k_n) and its suffix
     count stn[n,r] = sum_{m>n} rm8[m,r] (packed as one PSUM tile
     [stn | 4096*rm]).  Scatter targets and the last-writer gate:
        rpv[p]   = off_n + j + 1e6 + 1e6*(j >= k_n)
        a96c[p]  = stn96[p,r_p] + 4096*rm96[p,r_p]   # one-hot + accum_out
        w96[p]   = (a96c[p] == 4096)                 # covered, no later writer
        ridx[p]  = rpv[p] - 1e6*w96[p]
  3. One indirect DMA scatters x7 rows to out[ridx].  Gating makes the
     destinations UNIQUE (exactly the winning writer per row), so nothing
     relies on DMA descriptor ordering; indices >= 1e6 are skipped via
     bounds_check, leaving those rows at the zero-fill value.

All arithmetic is exact (masks are 0/1, indices are small ints), so the
output matches the reference bit-for-bit.  Every core runs the full
(tiny) computation; core 0's output is returned.  Measured on trn2:
~17.5 us HW exec per core (vs ~13.5 us for an empty DMA-through kernel
on this stack), relative error 0.0.
"""

import sys

import numpy as np

_TRN_REPO = "/opt/trn_rl_repo"
if _TRN_REPO not in sys.path:
    sys.path.insert(0, _TRN_REPO)

import ml_dtypes

import concourse.bacc as bacc
import concourse.bass as bass
import concourse.mybir as mybir
import concourse.tile as tile
from concourse.bass_utils import run_bass_kernel_spmd

B = 8          # batches
N_FULL = 8192  # detections per batch in the full input
J = 12         # num_dets < 12, so only rows [:12] of each batch matter
R = 100        # fixed output rows
P96 = B * J    # 96 stacked (batch, j) rows
OOB = 1.0e6    # pushed past bounds_check so the scatter skips the row

F32 = mybir.dt.float32
BF16 = mybir.dt.bfloat16
I32 = mybir.dt.int32

# f32 constant blob CB96 [96,3] = j96 | j96+OOB | vd96
CONST_LEN = P96 * 3
# bf16 constant blob: U96 | SEL96 | 4096*SEL96, packed per-row as [8,288]
GW = 4096.0  # weight separating the rm-half from the stn-half in the accum
CONSTBF_LEN = 8 * (3 * P96)


def _make_consts():
    p = np.arange(P96)
    m = np.arange(B)
    j96 = (p % J).astype(np.float32)[:, None]                            # [96,1]
    vd96 = (p // J).astype(np.float32)[:, None]                          # [96,1]
    blob = (
        np.concatenate([j96, j96 + OOB, vd96], axis=1).ravel().astype(np.float32)
    )
    assert blob.shape == (CONST_LEN,)
    u96 = (m[:, None] > p[None, :] // J).astype(np.float32)              # [8,96]
    sel96 = (m[:, None] == p[None, :] // J).astype(np.float32)           # [8,96]
    blobbf = (
        np.concatenate([u96, sel96, GW * sel96], axis=1)
        .ravel()
        .astype(ml_dtypes.bfloat16)
    )
    assert blobbf.shape == (CONSTBF_LEN,)
    return np.ascontiguousarray(blob), np.ascontiguousarray(blobbf)


def _build_nc() -> bass.Bass:
    nc = bacc.Bacc(None, target_bir_lowering=False, num_swdge_queues=4)
    nd_d = nc.dram_tensor("num_dets", [B], I32, kind="ExternalInput")
    boxes_d = nc.dram_tensor("boxes", [B, N_FULL, 4], F32, kind="ExternalInput")
    scores_d = nc.dram_tensor("scores", [B, N_FULL], F32, kind="ExternalInput")
    classes_d = nc.dram_tensor("classes", [B, N_FULL], F32, kind="ExternalInput")
    const_d = nc.dram_tensor("consts", [CONST_LEN], F32, kind="ExternalInput")
    constbf_d = nc.dram_tensor("constsbf", [CONSTBF_LEN], BF16, kind="ExternalInput")
    out_d = nc.dram_tensor("out", [R, 7], F32, kind="ExternalOutput")

    with tile.TileContext(nc) as tc:
        with (
            tc.tile_pool(name="sb", bufs=1) as sb,
            tc.tile_pool(name="ps", bufs=1, space=bass.MemorySpace.PSUM) as ps,
        ):
            ndi = sb.tile([B, 1], I32)
            cb96 = sb.tile([P96, 3], F32)
            r8i = sb.tile([B, R], I32)
            r2i = sb.tile([P96, 2 * R], I32)
            usel = sb.tile([B, 3 * P96], BF16)
            x7 = sb.tile([P96, 7], F32)
            z7 = sb.tile([R, 7], F32)

            k32 = sb.tile([32, 1], F32)
            off32 = sb.tile([32, 1], F32)
            k8bf = sb.tile([B, 1], BF16)
            off8bf = sb.tile([B, 1], BF16)
            s8f = sb.tile([B, 1], F32)
            u8c = sb.tile([B, R], F32)
            rm8 = sb.tile([B, R], BF16)
            b2 = sb.tile([P96, 1], F32)
            rpv = sb.tile([P96, 1], F32)
            scr200 = sb.tile([P96, 2 * R], F32)
            a96c = sb.tile([P96, 1], F32)
            w96 = sb.tile([P96, 1], F32)
            ridx = sb.tile([P96, 1], I32)

            comb = ps.tile([P96, 2 * R], F32)
            k96p = ps.tile([P96, 1], F32)
            off96p = ps.tile([P96, 1], F32)

            U96 = usel[:, 0:P96]
            SEL96 = usel[:, P96 : 2 * P96]
            SEL96W = usel[:, 2 * P96 : 3 * P96]
            J96 = cb96[:, 0:1]
            JO96 = cb96[:, 1:2]
            VD96 = cb96[:, 2:3]

            nc.gpsimd.memset(k32[:], 0.0)
            nc.gpsimd.memset(z7[:], 0.0)
            # on-device iotas replace the big row-index constants:
            # r8i[n,r] = r; r2i[p,:] = [r+OOB | r+OOB] (both accum halves)
            nc.gpsimd.iota(r8i[:], pattern=[[1, R]], base=0, channel_multiplier=0)
            nc.gpsimd.iota(
                r2i[:], pattern=[[0, 2], [1, R]], base=int(OOB), channel_multiplier=0
            )

            # loads on the two HWDGE queues only (gpsimd is kept free for the
            # indirect scatter); critical ones first.  The zero-fill of the
            # output is a plain DMA; an explicit dep orders it before the
            # scatter.
            nc.sync.dma_start(out=ndi[:], in_=nd_d[:].rearrange("(p f) -> p f", f=1))
            zfill = nc.gpsimd.dma_start(out=out_d[:], in_=z7[:])
            nc.sync.dma_start(out=x7[:, 1:5], in_=boxes_d[:, 0:J, :])
            nc.gpsimd.dma_start(out=x7[:, 5:6], in_=classes_d[:, 0:J])
            nc.scalar.dma_start(
                out=cb96[:], in_=const_d[:].rearrange("(p f) -> p f", p=P96)
            )
            nc.scalar.dma_start(out=usel[:], in_=constbf_d[:].rearrange(
                "(p f) -> p f", p=B
            ))
            nc.gpsimd.dma_start(out=x7[:, 6:7], in_=scores_d[:, 0:J])

            alu = mybir.AluOpType
            vec = nc.vector

            # k32[0:8] = float(num_dets); off32[n] = k32[n-1] via partition shift
            vec.tensor_copy(k32[0:B, :], ndi[:])
            vec.tensor_copy(k8bf[:], k32[0:B, :])
            vec.stream_shuffle(off32[:], k32[:], mask=[31] + list(range(31)))
            vec.tensor_copy(off8bf[:], off32[0:B, :])
            # vd column of x7 (scalar engine: DVE is the busy one)
            nc.scalar.copy(x7[:, 0:1], VD96)
            # broadcast k and off to the 96 (n,j) rows via tiny matmuls
            nc.tensor.matmul(k96p[:], SEL96, k8bf[:], start=True, stop=True)
            nc.tensor.matmul(off96p[:], SEL96, off8bf[:], start=True, stop=True)
            # batch coverage masks on 8 partitions
            vec.tensor_tensor(s8f[:], k32[0:B, :], off32[0:B, :], alu.add)
            vec.tensor_scalar(u8c[:], r8i[:], off32[0:B, :], None, alu.is_ge)
            vec.scalar_tensor_tensor(
                rm8[:], r8i[:], s8f[:], u8c[:], alu.is_lt, alu.mult
            )
            # two parallel matmuls into one PSUM tile: cols 0:100 hold
            # stn96[p,r] = sum_{m>n} rm8[m,r], cols 100:200 hold GW*rm8[n,r]
            nc.tensor.matmul(comb[:, 0:R], U96, rm8[:], start=True, stop=True)
            nc.tensor.matmul(comb[:, R : 2 * R], SEL96W, rm8[:], start=True, stop=True)

            # per-(n,j) scatter targets (fills DVE gaps while PE runs);
            # rpv = off + j + OOB + OOB*(j >= k)
            vec.tensor_scalar(b2[:], k96p[:], J96, OOB, alu.is_le, alu.mult)
            rpv_op = vec.scalar_tensor_tensor(
                rpv[:], off96p[:], JO96, b2[:], alu.add, alu.add
            )

            # one-hot extraction of both halves at r+OOB = rpv[p]:
            # a96c[p] = stn96[p,r_p] + GW*rm96[p,r_p]; winner iff == GW
            vec.scalar_tensor_tensor(
                scr200[:], r2i[:], rpv[:], comb[:], alu.is_equal, alu.mult,
                accum_out=a96c[:],
            )
            vec.tensor_scalar(w96[:], a96c[:], GW, None, alu.is_equal)
            # ridx = rpv - OOB*w96: winners land on their row, rest stay OOB
            vec.scalar_tensor_tensor(
                ridx[:], w96[:], -OOB, rpv[:], alu.mult, alu.add
            )

            # wake gpsimd early (on rpv) so its sequencer is already parked
            # at the scatter's own wait when ridx posts
            wk = nc.gpsimd.engine_nop()
            bass._add_dep_helper(wk.ins, rpv_op.ins, sync=True, reason="early wake")

            # winner-only scatter: destinations are unique, no ordering needed
            scat = nc.gpsimd.indirect_dma_start(
                out=out_d[:],
                out_offset=bass.IndirectOffsetOnAxis(ap=ridx[:], axis=0),
                in_=x7[:],
                in_offset=None,
                bounds_check=R - 1,
                oob_is_err=False,
            )
            # the zero-fill must fully land before the data scatter
            bass._add_dep_helper(
                scat.ins, zfill.ins, sync=True, reason="zero-fill before scatter"
            )

    nc.finalize()
    return nc


_CACHE: dict = {}


def _get_built():
    if "nc" not in _CACHE:
        _CACHE["nc"] = _build_nc()
        _CACHE["consts"] = _make_consts()
    return _CACHE["nc"], _CACHE["consts"]


def run(inputs: dict, trace: bool = False, **spmd_kwargs):
    """Run on all 8 cores with replicated inputs; returns (out, BassKernelResults)."""
    nc, (consts, constsbf) = _get_built()
    in_map = {
        "num_dets": np.ascontiguousarray(inputs["num_dets"], dtype=np.int32),
        "boxes": np.ascontiguousarray(inputs["boxes"], dtype=np.float32),
        "scores": np.ascontiguousarray(inputs["scores"], dtype=np.float32),
        "classes": np.ascontiguousarray(inputs["classes"], dtype=np.float32),
        "consts": consts,
        "constsbf": constsbf,
    }
    res = run_bass_kernel_spmd(
        nc,
        [dict(in_map) for _ in range(8)],
        core_ids=list(range(8)),
        trace=trace,
        **spmd_kwargs,
    )
    return res.results[0]["out"], res


def kernel(num_dets, boxes, scores, classes):
    out, _ = run(
        {"num_dets": num_dets, "boxes": boxes, "scores": scores, "classes": classes}
    )
    return out
